# revision 1
# baseline (speedup 1.0000x reference)
"""EMA kernel for Trainium2 (Bass/Tile), 8-core SPMD.

Problem: a[b, c, 0] = x[b, c, 0]
         a[b, c, t] = w[c] * x[b, c, t] + (1 - w[c]) * a[b, c, t-1]
         output[b, t, c] = a[b, c, t],  w = clip(weights, 0, 0.2)

Strategy (per core, B sharded 8 ways -> 8 batches/core):
  - x tile [128 chans (partitions), 2 halves, 2048 t]  (one 2MB DMA per batch)
  - Pool: wx = w * x                  (per-partition scalar multiply)
  - DVE : tensor_tensor_scan          state = (1-w)*state + wx, init = x[:, 0]
          -> the EMA recurrence in a single HW instruction per lane tile;
          (1-w) comes in as a stride-0 broadcast AP
  - PE  : transpose 128x128 blocks -> PSUM [t, c] (2-bank tiles, 8 blocks)
  - ACT/DVE: copy PSUM -> SBUF staging [128 t, 4096]
  - DMA out 2MB per store, 1KB contiguous rows of [t, c]
"""

from contextlib import ExitStack

import numpy as np

import concourse.bass as bass
import concourse.tile as tile
from concourse import mybir
from concourse.bass_utils import run_bass_kernel_spmd

B, C, T = 64, 256, 2048
N_CORES = 8
B_LOC = B // N_CORES  # 8 batches per core
P = 128
NH = C // P  # 2 channel halves
NTB = T // P  # 16 time blocks
F32 = mybir.dt.float32


def build_nc():
    nc = bass.Bass()
    x = nc.dram_tensor("x", [B_LOC, C, T], F32, kind="ExternalInput")
    # wtab columns: [w_h0, w_h1, (1-w)_h0, (1-w)_h1]
    wtab = nc.dram_tensor("wtab", [P, 4], F32, kind="ExternalInput")
    ident = nc.dram_tensor("ident", [P, P], F32, kind="ExternalInput")
    out = nc.dram_tensor("out", [B_LOC, T, C], F32, kind="ExternalOutput")

    with tile.TileContext(nc) as tc, ExitStack() as ctx:
        consts = ctx.enter_context(tc.tile_pool(name="consts", bufs=1))
        xp = ctx.enter_context(tc.tile_pool(name="xp", bufs=4))
        apool = ctx.enter_context(tc.tile_pool(name="apool", bufs=9))
        stage = ctx.enter_context(tc.tile_pool(name="stage", bufs=3))
        psum = ctx.enter_context(tc.tile_pool(name="psum", bufs=2, space="PSUM"))

        id_t = consts.tile([P, P], F32)
        nc.scalar.dma_start(out=id_t, in_=ident[:, :])
        wt = consts.tile([P, 4], F32)
        nc.scalar.dma_start(out=wt, in_=wtab[:, :])

        for pair in range(B_LOC // 2):
            a_tiles = []  # index g = bb*2 + h
            for bb in range(2):
                b = pair * 2 + bb
                x_t = xp.tile([P, NH, T], F32, tag="x")
                xr = x[b].rearrange("(h p) t -> p h t", p=P)
                for h2 in range(NH):
                    # ACT-issued HWDGE: keeps load descriptor-gen out of the
                    # SP stream so stores aren't queued behind it. The very
                    # first load-half goes on SP so its generation overlaps
                    # the ACT-issued consts.
                    if pair == 0 and h2 == 0:
                        nc.sync.dma_start(out=x_t[:, h2, :], in_=xr[:, h2, :])
                    else:
                        nc.scalar.dma_start(out=x_t[:, h2, :], in_=xr[:, h2, :])
                for h in range(NH):
                    a_t = apool.tile([P, T], F32, tag="a")
                    nc.gpsimd.tensor_scalar_mul(
                        out=a_t, in0=x_t[:, h, :], scalar1=wt[:, h : h + 1]
                    )
                    # EMA: state = (1-w)*state + w*x ; out[0]=x[0] via init
                    nc.vector.tensor_tensor_scan(
                        out=a_t,
                        data0=wt[:, 2 + h : 3 + h].to_broadcast((P, T)),
                        data1=a_t,
                        initial=x_t[:, h, 0:1],
                        op0=mybir.AluOpType.mult,
                        op1=mybir.AluOpType.add,
                    )
                    a_tiles.append(a_t)

            for tbg in range(2):  # halves of T
                st = stage.tile([P, 8 * 4 * P], F32)  # [128, 4096]
                for tp in range(4):  # 2-bank psum tiles, 2 t-blocks each
                    ps = psum.tile([P, 2 * 4 * P], F32)  # [128, 1024]
                    for sub in range(2):
                        tb = tbg * 8 + tp * 2 + sub
                        for g in range(4):
                            nc.tensor.transpose(
                                ps[:, sub * 512 + g * P : sub * 512 + (g + 1) * P],
                                a_tiles[g][:, tb * P : (tb + 1) * P],
                                id_t,
                            )
                    nc.any.tensor_copy(
                        out=st[:, tp * 1024 : (tp + 1) * 1024], in_=ps
                    )
                # 1MB store per batch: 8 t-blocks x 256 chans, 1KB rows
                st4 = st.rearrange("p (tb bb c) -> p tb bb c", tb=8, bb=2)
                for bb in range(2):
                    dst = out[
                        pair * 2 + bb, tbg * 1024 : (tbg + 1) * 1024, :
                    ].rearrange("(tb p) c -> p tb c", p=P)
                    nc.sync.dma_start(out=dst, in_=st4[:, :, bb, :])

    split_excess_waits(nc)
    return nc


def split_excess_waits(nc, cap=1):
    """Hoist all but `cap` semaphore waits of each instruction into standalone
    EventSemaphore instructions right before it (walrus's setupSyncWait only
    encodes one wait per TPB instruction)."""
    n_split = 0
    for f in nc.m.functions:
        for blk in f.blocks:
            new_insts = []
            for ins in blk.instructions:
                si = ins.sync_info
                waits = list(si.on_wait) if si and si.on_wait else []
                if len(waits) > cap:
                    for i, w in enumerate(waits[:-cap]):
                        es = mybir.InstEventSemaphore(
                            name=f"{ins.name}-esw{i}", ins=[], outs=[]
                        )
                        es.engine = ins.engine
                        es.sync_info = mybir.SyncInfo(on_wait=[w], on_update=[])
                        new_insts.append(es)
                        n_split += 1
                    ins.sync_info = mybir.SyncInfo(
                        on_wait=waits[-cap:], on_update=si.on_update
                    )
                new_insts.append(ins)
            blk.instructions = new_insts
    return n_split


_NC_CACHE = []


def _get_nc():
    if not _NC_CACHE:
        _NC_CACHE.append(build_nc())
    return _NC_CACHE[0]


def _make_in_maps(x, weights):
    x = np.ascontiguousarray(np.asarray(x, dtype=np.float32))
    w = np.clip(np.asarray(weights, dtype=np.float32), 0.0, 0.2).astype(np.float32)
    onemw = (np.float32(1.0) - w).astype(np.float32)
    wtab = np.stack(
        [w[:P], w[P:], onemw[:P], onemw[P:]], axis=1
    )  # [128, 4]
    wtab = np.ascontiguousarray(wtab, dtype=np.float32)
    ident = np.eye(P, dtype=np.float32)
    return [
        {
            "x": np.ascontiguousarray(x[i * B_LOC : (i + 1) * B_LOC]),
            "wtab": wtab,
            "ident": ident,
        }
        for i in range(N_CORES)
    ]


def run(x, weights, **run_kwargs):
    nc = _get_nc()
    res = run_bass_kernel_spmd(
        nc, _make_in_maps(x, weights), core_ids=list(range(N_CORES)), **run_kwargs
    )
    full = np.concatenate([r["out"] for r in res.results], axis=0)
    return full, res


def kernel(x, initial_state=None, weights=None):
    # initial_state is accepted but unused (matches the reference module).
    full, _ = run(x, weights)
    return full



# revision 24
# speedup vs baseline: 1.8642x; 1.8642x over previous
"""EMA kernel for Trainium2 (Bass/Tile), 8-core SPMD, bf16 I/O.

Problem: a[b, c, 0] = x[b, c, 0]
         a[b, c, t] = w[c] * x[b, c, t] + (1 - w[c]) * a[b, c, t-1]
         output[b, t, c] = a[b, c, t],  w = clip(weights, 0, 0.2)

The kernel is HBM-bandwidth bound (134MB in + 134MB out at 360GB/s/core
across 8 cores). Both sides of the HBM traffic are bf16 (converted on
host), halving DMA time vs f32; the rel-err budget (2e-2) dwarfs the
~2e-3 this costs.

Compute is arranged so the only scan-capable engine (DVE) does nothing
but the recurrence, and every other engine stays well under the DMA
bound:
  - Rescaled recurrence: s_t = (1-w)*s_{t-1} + x_t with s_0 = x_0/w,
    so a_t = w*s_t. This removes the big per-tile premultiply w*x;
    only a [128,1] seed multiply per tile remains (DVE, inline, so the
    seed can never stall the scan pipeline behind other engines' work).
  - DVE tensor_tensor_scan runs the recurrence with fp32 internal
    state ((1-w) stays fp32; only the out stream is bf16), so there is
    no error accumulation along T.
  - The final a = w*s is folded into the PE transpose: the "identity"
    operand is diag(w) (bf16), making the [c,t]->[t,c] transpose apply
    the per-channel scale for free.
  - PSUM -> SBUF staging copies are split Pool/ACT; stores go out with
    512B contiguous rows (the full-rate DMA descriptor threshold).
  - Loads for the last two batches are issued late (after mid-stream
    copies) so the DMA engines keep load work in reserve for the tail,
    where stores alone arrive slower than the DMA drains them.
"""

from contextlib import ExitStack

import ml_dtypes
import numpy as np

import concourse.bass as bass
import concourse.tile as tile
from concourse import mybir
from concourse.bass_utils import run_bass_kernel_spmd

B, C, T = 64, 256, 2048
N_CORES = 8
B_LOC = B // N_CORES  # 8 batches per core
P = 128
NH = C // P  # 2 channel halves
NTB = T // P  # 16 time blocks
F32 = mybir.dt.float32
BF16 = mybir.dt.bfloat16
NP_BF16 = np.dtype(ml_dtypes.bfloat16)

# batch -> loop iteration at whose start its loads are issued (late loads
# keep the DMA queue fed through the tail). Others load up front.
DEFERRED_LOADS = {6: 4, 7: 5}


def build_nc():
    nc = bass.Bass()
    x = nc.dram_tensor("x", [B_LOC, C, T], BF16, kind="ExternalInput")
    # wtab columns: [winv_h0, winv_h1, (1-w)_h0, (1-w)_h1]
    wtab = nc.dram_tensor("wtab", [P, 4], F32, kind="ExternalInput")
    # wdiag[p, h, :] = w[h*128+p] * e_p  (diag(w) per channel half)
    wdiag = nc.dram_tensor("wdiag", [P, NH, P], BF16, kind="ExternalInput")
    out = nc.dram_tensor("out", [B_LOC, T, C], BF16, kind="ExternalOutput")

    with tile.TileContext(nc) as tc, ExitStack() as ctx:
        consts = ctx.enter_context(tc.tile_pool(name="consts", bufs=1))
        xp = ctx.enter_context(tc.tile_pool(name="xp", bufs=B_LOC))
        s0p = ctx.enter_context(tc.tile_pool(name="s0p", bufs=4))
        apool = ctx.enter_context(tc.tile_pool(name="apool", bufs=6))
        stage = ctx.enter_context(tc.tile_pool(name="stage", bufs=6))
        psum = ctx.enter_context(tc.tile_pool(name="psum", bufs=4, space="PSUM"))

        wd_t = consts.tile([P, NH, P], BF16)
        nc.scalar.dma_start(out=wd_t, in_=wdiag[:, :, :])
        wt = consts.tile([P, 4], F32)
        nc.scalar.dma_start(out=wt, in_=wtab[:, :])

        x_tiles = {}

        def load_batch(b):
            x_t = xp.tile([P, NH, T], BF16, tag="x")
            xr = x[b].rearrange("(h p) t -> p h t", p=P)
            for h2 in range(NH):
                # ACT-issued HWDGE: keeps load descriptor-gen out of the
                # SP stream so stores aren't queued behind it. The very
                # first load-half goes on SP so its generation overlaps
                # the ACT-issued consts.
                if b == 0 and h2 == 0:
                    nc.sync.dma_start(out=x_t[:, h2, :], in_=xr[:, h2, :])
                else:
                    nc.scalar.dma_start(out=x_t[:, h2, :], in_=xr[:, h2, :])
            x_tiles[b] = x_t

        for b in range(B_LOC):
            if b not in DEFERRED_LOADS:
                load_batch(b)

        for b in range(B_LOC):
            for db, when in DEFERRED_LOADS.items():
                if when == b:
                    load_batch(db)
            x_t = x_tiles[b]
            a_tiles = []  # index h
            for h in range(NH):
                # s_0 = x_0 / w. Seeds run on Pool, which does nothing else:
                # its in-order queue then never parks a seed behind slow
                # downstream work, so the scan pipeline cannot stall on it.
                s0_t = s0p.tile([P, 1], F32, tag="s0")
                nc.gpsimd.tensor_scalar_mul(
                    out=s0_t, in0=x_t[:, h, 0:1], scalar1=wt[:, h : h + 1]
                )
                # s_t = (1-w)*s_{t-1} + x_t ; fp32 state, bf16 out.
                # (The scan opcode is DVE-only: walrus rejects it on Pool.)
                a_t = apool.tile([P, T], BF16, tag="a")
                nc.vector.tensor_tensor_scan(
                    out=a_t,
                    data0=wt[:, 2 + h : 3 + h].to_broadcast((P, T)),
                    data1=x_t[:, h, :],
                    initial=s0_t,
                    op0=mybir.AluOpType.mult,
                    op1=mybir.AluOpType.add,
                )
                a_tiles.append(a_t)

            for tbg in range(2):  # halves of T
                st = stage.tile([P, 8 * 2 * P], BF16)  # [128, 2048]
                for tpp in range(2):  # 2-bank psum tiles, 4 t-blocks each
                    ps = psum.tile([P, 4 * 2 * P], F32)  # [128, 1024]
                    for q in range(2):
                        for sub in range(2):
                            tb = tbg * 8 + tpp * 4 + q * 2 + sub
                            for h in range(NH):
                                # Scaled transpose as a REGULAR matmul:
                                # out = lhsT.T @ rhs with lhsT the [c,t]
                                # s-block and rhs = diag(w) gives
                                # out[t,c] = w[c]*s[c,t]. (The special
                                # transpose datapath does NOT apply the
                                # matrix values on HW, so it cannot carry
                                # the w scale; a plain matmul does.)
                                nc.tensor.matmul(
                                    ps[
                                        :,
                                        q * 512
                                        + sub * 256
                                        + h * P : q * 512
                                        + sub * 256
                                        + (h + 1) * P,
                                    ],
                                    a_tiles[h][:, tb * P : (tb + 1) * P],
                                    wd_t[:, h, :],
                                    start=True,
                                    stop=True,
                                )
                    # PSUM -> SBUF staging on ACT. Pool cannot access PSUM
                    # (BIR verifier), and DVE must stay free for scans —
                    # except the last batch, whose copies run after the
                    # final scan when DVE is idle and fastest (2-byte mode).
                    dst = st[:, tpp * 1024 : (tpp + 1) * 1024]
                    if b == B_LOC - 1:
                        nc.vector.tensor_copy(out=dst, in_=ps)
                    else:
                        nc.scalar.copy(out=dst, in_=ps)
                # 512KB store per batch-half: 8 t-blocks x 256 chans,
                # 512B rows
                st3 = st.rearrange("p (tb c) -> p tb c", tb=8)
                dst = out[
                    b, tbg * 1024 : (tbg + 1) * 1024, :
                ].rearrange("(tb p) c -> p tb c", p=P)
                nc.sync.dma_start(out=dst, in_=st3)

    split_excess_waits(nc)
    return nc


def split_excess_waits(nc, cap=1):
    """Hoist all but `cap` semaphore waits of each instruction into standalone
    EventSemaphore instructions right before it (walrus's setupSyncWait only
    encodes one wait per TPB instruction)."""
    n_split = 0
    for f in nc.m.functions:
        for blk in f.blocks:
            new_insts = []
            for ins in blk.instructions:
                si = ins.sync_info
                waits = list(si.on_wait) if si and si.on_wait else []
                if len(waits) > cap:
                    for i, w in enumerate(waits[:-cap]):
                        es = mybir.InstEventSemaphore(
                            name=f"{ins.name}-esw{i}", ins=[], outs=[]
                        )
                        es.engine = ins.engine
                        es.sync_info = mybir.SyncInfo(on_wait=[w], on_update=[])
                        new_insts.append(es)
                        n_split += 1
                    ins.sync_info = mybir.SyncInfo(
                        on_wait=waits[-cap:], on_update=si.on_update
                    )
                new_insts.append(ins)
            blk.instructions = new_insts
    return n_split


_NC_CACHE = []


def _get_nc():
    if not _NC_CACHE:
        _NC_CACHE.append(build_nc())
    return _NC_CACHE[0]


def _make_in_maps(x, weights):
    x = np.asarray(x, dtype=np.float32).astype(NP_BF16)
    x = np.ascontiguousarray(x)
    w = np.clip(np.asarray(weights, dtype=np.float32), 0.0, 0.2).astype(np.float32)
    # Floor w away from 0 so 1/w stays finite. For w below the floor the
    # reference output is constant x_0 per lane, which the rescaled
    # recurrence reproduces (the running sum is negligible vs x_0/w).
    w_safe = np.maximum(w, np.float32(1e-12))
    winv = (np.float32(1.0) / w_safe).astype(np.float32)
    onemw = (np.float32(1.0) - w).astype(np.float32)
    wtab = np.stack(
        [winv[:P], winv[P:], onemw[:P], onemw[P:]], axis=1
    )  # [128, 4]
    wtab = np.ascontiguousarray(wtab, dtype=np.float32)
    wdiag = np.zeros((P, NH, P), dtype=NP_BF16)
    for h in range(NH):
        wdiag[np.arange(P), h, np.arange(P)] = w_safe[h * P : (h + 1) * P].astype(
            NP_BF16
        )
    return [
        {
            "x": np.ascontiguousarray(x[i * B_LOC : (i + 1) * B_LOC]),
            "wtab": wtab,
            "wdiag": wdiag,
        }
        for i in range(N_CORES)
    ]


def run(x, weights, **run_kwargs):
    nc = _get_nc()
    res = run_bass_kernel_spmd(
        nc, _make_in_maps(x, weights), core_ids=list(range(N_CORES)), **run_kwargs
    )
    full = np.concatenate(
        [np.asarray(r["out"]).astype(np.float32) for r in res.results], axis=0
    )
    return full, res


def kernel(x, initial_state=None, weights=None):
    # initial_state is accepted but unused (matches the reference module).
    full, _ = run(x, weights)
    return full
